# revision 19
# baseline (speedup 1.0000x reference)
"""Trainium2 Bass kernel for nn_Attention_49185965473844.

Math (per example b):
    q = x @ Wq ; k = x @ Wk ; v = x @ Wv          (x: [S, D], W*: [D, D], D=32)
    A[q,k]   = sum_s q[s,q] k[s,k]  = (Wq^T G Wk)[q,k],   G = x^T x   ([32, 32])
    scores   = softmax(A, axis=q)                 (normalize down columns)
    out[q,s] = sum_k scores[q,k] v[s,k] = (M @ x^T)[q,s], M = scores @ Wv^T

So the whole problem reduces to: one Gram matrix G = x^T x per example (the
only big contraction, streamed over S), a tiny 32x32 chain + softmax, and one
[32,32] @ [32,S] matmul against x^T (PE transposes of the resident x tile).

Sharding: pure data parallel over batch B=64 -> 8 examples per NeuronCore.
"""

import numpy as np

import concourse.bass as bass
import concourse.bacc as bacc
import concourse.tile as tile
from concourse import mybir
from concourse.bass_utils import run_bass_kernel_spmd

N_CORES = 8
B, S, D = 64, 8192, 32
PER_CORE = B // N_CORES  # 8

F32 = mybir.dt.float32
F32R = mybir.dt.float32r

# float32r (TF32-like reduced-precision PE mode, 1 cyc/row at moving dim
# >= 256 vs fp32's 4) for the Gram, transpose and output matmuls. Inputs
# must be explicitly rounded by their producer; the rounding rides existing
# copies (PSUM->SBUF) plus one GPSIMD pass over x (idle engine, line rate).
USE_F32R = True


def build_nc(n_ex=PER_CORE, seq=S):
    """Build the per-core Bass program. Same program runs on all 8 cores.

    s-index decomposition: s = 128*c + p, chunk c = 4*t + j (quad t, partition
    block j), quad t = 8*g + h (store group g).  So
        s = 4096*g + 512*h + 128*j + p,   h in [0,8), j in [0,4), p in [0,128).
    The PE transpose of natural-tile quad t produces partition (j, d), free p;
    the final matmul then yields out rows (j, q) and free (h, p) per group —
    stored with one 3-dim DMA per (g, j).
    """
    assert seq % 2048 == 0
    n_chunks = seq // 128     # 128-row chunks of x
    n_quads = n_chunks // 4   # [128, 128] column blocks of the natural tile
    n_groups = n_quads // 4   # store groups: 4 quads -> [128, 512] out tiles

    nc = bacc.Bacc("TRN2", target_bir_lowering=False, debug=False)
    x_t = nc.declare_dram_parameter("x", [n_ex, seq, D], F32, isOutput=False)
    cst_t = nc.declare_dram_parameter("cst", [128, 352], F32, isOutput=False)
    out_t = nc.declare_dram_parameter("out", [n_ex, D, seq], F32, isOutput=True)

    with tile.TileContext(nc) as tc:
        with (
            tc.tile_pool(name="consts", bufs=1) as consts,
            tc.tile_pool(name="nat_pool", bufs=4) as nat_pool,
            tc.tile_pool(name="trhs_pool", bufs=6) as trhs_pool,
            tc.tile_pool(name="osb_pool", bufs=n_ex * n_groups) as osb_pool,
            tc.tile_pool(name="small_pool", bufs=3) as small_pool,
            tc.tile_pool(name="acc_psum", bufs=2, space="PSUM") as acc_psum,
            tc.tile_pool(name="tp_psum", bufs=3, space="PSUM") as tp_psum,
            tc.tile_pool(name="o_psum", bufs=3, space="PSUM") as o_psum,
        ):
            # ---- constants: one DMA so every PE consumer has a single
            # upstream sync (fp32 matmuls only get 1 sync wait in walrus) ----
            cst_sb = consts.tile([128, 352], F32)
            nc.sync.dma_start(out=cst_sb, in_=cst_t[:, :])
            identity = cst_sb[:, 0:128]
            wv4 = cst_sb[:, 128:160]       # np.tile(Wv, (4, 1))
            wq_sb = cst_sb[0:D, 160:192]
            wk_sb = cst_sb[0:D, 192:224]
            blkmask = cst_sb[:, 224:352]   # [p, c] = 1.0 iff p//32 == c//32
            # Wv replicated on 4 partition blocks, PE-transposed so that
            # wvt_rep[k, 32*j + d] = Wv[d, k].
            wvt_ps = acc_psum.tile([D, 128], F32, tag="acc")
            nc.tensor.transpose(wvt_ps, wv4, identity)
            wvt_rep = consts.tile([D, 128], F32)
            nc.scalar.copy(out=wvt_rep, in_=wvt_ps)

            def make_tp(nat2, g):
                """PE-transpose quads t = 4g + (0..3) into one PSUM bank."""
                tp_ps = tp_psum.tile(
                    [128, 512], F32, tag="tp", name=f"tp_{g}"
                )
                for i in range(4):
                    t = 4 * g + i
                    nc.tensor.transpose(
                        tp_ps[:, 128 * i:128 * (i + 1)],
                        nat2[:, 128 * t:128 * (t + 1)],
                        identity,
                    )
                return tp_ps

            def load_nat(b):
                # x_b as [128, n_chunks * 32]; chunk c col-block holds
                # x[128*c + p, :] on partition p. Two half-loads so the Gram
                # matmuls can start after the first half lands.
                nat = nat_pool.tile([128, n_chunks, D], F32, tag="nat",
                                    name=f"nat_{b}")
                half = n_chunks // 2
                src_ap = x_t[b].rearrange("(c p) d -> p c d", p=128)
                nc.sync.dma_start(out=nat[:, 0:half, :], in_=src_ap[:, 0:half, :])
                nc.sync.dma_start(out=nat[:, half:, :], in_=src_ap[:, half:, :])
                return nat

            # Loads are issued one example ahead of the store traffic that
            # shares the sync HWDGE ring, so descriptor generation for load
            # b+1 is never queued behind stores still waiting on example b's
            # tail compute.
            nats = {0: load_nat(0)}
            for b in range(n_ex):
                if b + 1 < n_ex:
                    nats[b + 1] = load_nat(b + 1)
                nat = nats.pop(b)
                nat2 = nat.rearrange("p c d -> p (c d)")

                # ---- Gram accumulation: 128x128 of quad cross-products;
                # the 4 diagonal 32x32 blocks sum to G = x^T x ----
                gram_ps = acc_psum.tile([128, 128], F32, tag="acc")
                for t in range(n_quads):
                    blk = nat2[:, 128 * t:128 * (t + 1)]
                    nc.tensor.matmul(
                        gram_ps,
                        lhsT=blk,
                        rhs=blk,
                        start=(t == 0),
                        stop=(t == n_quads - 1),
                    )

                # PE does group-0/1 transposes while ACT folds gram to SBUF.
                tp_tiles = {}
                tp_tiles[0] = make_tp(nat2, 0)
                tp_tiles[1] = make_tp(nat2, 1)

                # ---- fold the 4 diagonal 32x32 blocks of gram into G ----
                gram_sb = small_pool.tile([128, 128], F32, tag="gram_sb")
                nc.scalar.copy(out=gram_sb, in_=gram_ps[:, 0:128])
                g_ps = acc_psum.tile([D, D], F32, tag="acc")
                for j in range(4):
                    nc.tensor.matmul(
                        g_ps,
                        lhsT=identity[:, 32 * j:32 * (j + 1)],
                        rhs=gram_sb[:, 32 * j:32 * (j + 1)],
                        start=(j == 0),
                        stop=(j == 3),
                    )
                g_sb = small_pool.tile([D, D], F32, tag="g_sb")
                nc.scalar.copy(out=g_sb, in_=g_ps)

                # ---- A^T = Wk^T (G Wq);  G symmetric so lhsT=G works ----
                t2_ps = acc_psum.tile([D, D], F32, tag="acc")
                nc.tensor.matmul(t2_ps, lhsT=g_sb, rhs=wq_sb)
                t2_sb = small_pool.tile([D, D], F32, tag="t2_sb")
                nc.scalar.copy(out=t2_sb, in_=t2_ps)
                at_ps = acc_psum.tile([D, D], F32, tag="acc")
                nc.tensor.matmul(at_ps, lhsT=wk_sb, rhs=t2_sb)

                # ---- softmax over q (free dim of A^T), on DVE/ACT while the
                # PE runs the remaining transposes ----
                nmax = small_pool.tile([D, 1], F32, tag="nmax")
                nc.vector.reduce_max(
                    out=nmax, in_=at_ps, axis=mybir.AxisListType.X, negate=True
                )
                e_sb = small_pool.tile([D, D], F32, tag="e_sb")
                nc.scalar.activation(
                    out=e_sb, in_=at_ps,
                    func=mybir.ActivationFunctionType.Exp,
                    bias=nmax, scale=1.0,
                )
                rsum = small_pool.tile([D, 1], F32, tag="rsum")
                nc.vector.reduce_sum(out=rsum, in_=e_sb, axis=mybir.AxisListType.X)
                rinv = small_pool.tile([D, 1], F32, tag="rinv")
                nc.vector.reciprocal(out=rinv, in_=rsum)
                sc_sb = small_pool.tile([D, D], F32, tag="sc_sb")
                nc.vector.tensor_scalar_mul(out=sc_sb, in0=e_sb, scalar1=rinv)

                for g in range(2, n_groups):
                    tp_tiles[g] = make_tp(nat2, g)

                # ---- M^T replicated on 4 partition blocks ----
                m4_ps = acc_psum.tile([128, D], F32, tag="acc")
                nc.tensor.matmul(m4_ps, lhsT=wvt_rep, rhs=sc_sb)
                m4_sb = small_pool.tile([128, D], F32, tag="m4_sb")
                nc.scalar.copy(out=m4_sb, in_=m4_ps)
                # Block-diagonal lhsT for the output matmuls: one full-width
                # matmul per group instead of four 32x32 sub-tile matmuls
                # (walrus rejects f32r + tile_position). The mask multiply
                # also performs the f32r rounding.
                bd = small_pool.tile([128, 128], F32R if USE_F32R else F32,
                                     tag="bd")
                m4_bcast = bass.AP(
                    tensor=m4_sb.tensor,
                    offset=m4_sb.offset,
                    ap=[list(m4_sb.ap[0]), [0, 4], list(m4_sb.ap[1])],
                )
                nc.gpsimd.tensor_mul(
                    out=bd.rearrange("p (r q) -> p r q", r=4),
                    in0=m4_bcast,
                    in1=blkmask.rearrange("p (r q) -> p r q", r=4),
                )

                # ---- out tiles: 4 concurrent 32x32-subtile matmuls ----
                # Copy engine alternates by group parity (even: ACT, odd: DVE)
                # so each O-matmul's upstream ticks collapse onto one
                # semaphore while both engines share the copy volume.
                for g in range(n_groups):
                    trhs = trhs_pool.tile([128, 512], F32R if USE_F32R else F32,
                                          tag="trhs")
                    tp_src = tp_tiles[g].bitcast(F32R) if USE_F32R else tp_tiles[g]
                    if g % 2 == 0:
                        nc.scalar.copy(out=trhs, in_=tp_src)
                    else:
                        nc.vector.tensor_copy(out=trhs, in_=tp_src)
                    o_ps = o_psum.tile([128, 512], F32, tag="o")
                    nc.tensor.matmul(o_ps, lhsT=bd, rhs=trhs)
                    o_sb = osb_pool.tile([128, 512], F32, tag="o_sb")
                    if g % 2 == 0:
                        nc.scalar.copy(out=o_sb, in_=o_ps)
                    else:
                        nc.vector.tensor_copy(out=o_sb, in_=o_ps)
                    # o_sb[(j,q), (h,p)] = out_b[q, 2048 g + 512 h + 128 j + p];
                    # one 3-dim store per (g, j).
                    dst4 = out_t[b].rearrange(
                        "q (gg h j p) -> gg j q h p",
                        gg=n_groups, h=4, j=4, p=128,
                    )[g]
                    steng = nc.scalar if g % 2 == 0 else nc.sync
                    for j in range(4):
                        eng = steng if j < 2 else nc.gpsimd
                        eng.dma_start(
                            out=dst4[j], in_=o_sb[32 * j:32 * (j + 1), :]
                        )

    nc.compile()
    return nc


_CACHED_NC = None


def _get_nc():
    global _CACHED_NC
    if _CACHED_NC is None:
        _CACHED_NC = build_nc()
    return _CACHED_NC


def make_cst(wq, wk, wv):
    """[128, 352]: identity | tile(Wv,(4,1)) | Wq | Wk | 32x32 block mask."""
    cst = np.zeros((128, 352), dtype=np.float32)
    cst[:, 0:128] = np.eye(128, dtype=np.float32)
    cst[:, 128:160] = np.tile(wv, (4, 1))
    cst[0:D, 160:192] = wq
    cst[0:D, 192:224] = wk
    blk = np.arange(128) // 32
    cst[:, 224:352] = (blk[:, None] == blk[None, :]).astype(np.float32)
    return cst


def kernel(x, Wq, Wk, Wv):
    x = np.ascontiguousarray(np.asarray(x, dtype=np.float32))
    wq = np.asarray(Wq, dtype=np.float32).reshape(D, D)
    wk = np.asarray(Wk, dtype=np.float32).reshape(D, D)
    wv = np.asarray(Wv, dtype=np.float32).reshape(D, D)
    assert x.shape == (B, S, D)
    cst = make_cst(wq, wk, wv)

    nc = _get_nc()
    in_maps = [
        {
            "x": x[c * PER_CORE:(c + 1) * PER_CORE],
            "cst": cst,
        }
        for c in range(N_CORES)
    ]
    res = run_bass_kernel_spmd(nc, in_maps, list(range(N_CORES)))
    out = np.concatenate([res.results[c]["out"] for c in range(N_CORES)], axis=0)
    return out
